# revision 5
# baseline (speedup 1.0000x reference)
"""LinearQuant kernel for Trainium2 (8 NeuronCores, data parallel).

Reference math (fp32, bit-exact):
    delta = 2^-4; bound = 128
    out = clip(floor(x/delta + 0.5), -128, 127) * delta

Computed on-device with ONLY tensor_scalar-class ops (TT/STT ops measured
~4.5x slower than 2x-mode TS on this hardware, so the classic
RNE+compare-fixup floor was redesigned into an integer-domain floor):

  w = fl(fl(x + 2^-5) - 2^-6)        # u = fl(x+2^-5) = fl(16x+.5)/16 (pow2
                                     # scaling commutes with rounding); the
                                     # -2^-6 bias is EXACT for |u| <= 8
                                     # (span fits 24-bit mantissa)
  c = fl(w + 1.5*2^18)               # magic: c's low bits = K + k where
                                     # k = RNE(32w) = RNE(2y-0.5), ties-even
  s = c.bits >> 1                    # floor(v) == RNE(2v-0.5) >> 1 exactly
                                     # (incl. ties & negatives)
  a = s.bits_as_fp32 * 2^78 - 1.75*2^23  # -> the int index a = floor(16x+.5),
                                     # int8 out (ACT engine)

s.bits = 0x24600000 + a, i.e. fp32 value 1.75*2^-55 + a*2^-78; the *2^78
- 14680064 rebias is exact (pow2 scale; the subtract lands in [2^23,2^24)
where the grid is 1.0, and the result a is fp32-representable). The fp32
-> int8 output conversion of the exact integer a is exact, and its
saturation at [-128, 127] IS the reference's post-floor clip, so the
kernel is bit-exact for arbitrary inputs (verified on HW incl. ties and
out-of-range values). Storing int8 instead of bf16 halves store traffic:
per-core HBM is 25.7 MB in + 6.4 MB out, ~90us at the 358 GB/s
HBM-per-core limit -- the roofline. The host converts with a*2^-4
(exact pow2 scale).

Engine split: DVE runs w/c/s as fused TS ops (2x_2P mode, 2 elem/cyc
fp32); ACT runs the final rebias->int8 and the out-DMA triggers (own
HWDGE ring, so out never blocks the in stream); SP(sync) runs the
in-DMAs. Raw Block style with explicit semaphores (Tile's auto-sems hit
walrus "Too many sync wait commands" on this shape). The DVE stream is
software-pipelined (w(i), c(i-1), s(i-2)) over ring buffers.

Layout: each core's 6,422,528-elem flat slice is viewed as [128, 50176]
partition-major (a free reshape on host and an elementwise-consistent
relabeling on device). Chunks are column ranges, TAPERED: small chunks
at the head (so the first DVE op starts ~3us in instead of waiting for
a full 1.8 MB tile) and at the tail (short final act+store drain), 3584
in the middle. The xt ring has 4 slots for deeper DMA prefetch.
"""

import os

import numpy as np

B, C, H, W = 64, 256, 56, 56
N_CORES = 8
P = 128            # partitions
TOT = 50176        # free elems per partition per core
FMAX = 3584        # ring slot width (elems)
CHUNKS = ([256, 256, 512, 1024, 2048] + [3584] * 11
          + [1792, 1792, 1024, 1024, 512, 256, 256])
assert sum(CHUNKS) == TOT and max(CHUNKS) == FMAX
NCH = len(CHUNKS)
OFFS = [sum(CHUNKS[:i]) for i in range(NCH)]
M5 = 393216.0        # 1.5*2^18: RNE-magic for the 2^-5 grid
REBIAS = -14680064.0  # -1.75*2^23
SCALE78 = float(2.0 ** 78)

_cache = {}


def _build():
    from contextlib import ExitStack

    import concourse.mybir as mybir
    from concourse.bass import Bass

    fp32 = mybir.dt.float32
    int8 = mybir.dt.int8
    int32 = mybir.dt.int32
    alu = mybir.AluOpType
    act = mybir.ActivationFunctionType

    nc = Bass()
    xin = nc.declare_dram_parameter("x", [P, TOT], fp32, isOutput=False)
    yout = nc.declare_dram_parameter("y", [P, TOT], int8, isOutput=True)

    with ExitStack() as ctx:
        block = ctx.enter_context(nc.Block())
        s_in = [ctx.enter_context(nc.semaphore(f"s_in{j}")) for j in range(3)]
        s_out = [ctx.enter_context(nc.semaphore(f"s_out{j}")) for j in range(6)]
        s_w = ctx.enter_context(nc.semaphore("s_w"))      # DVE w ops done
        s_c2 = ctx.enter_context(nc.semaphore("s_c2"))    # DVE c ops done
        s_s = ctx.enter_context(nc.semaphore("s_s"))      # DVE s ops done
        s_f = ctx.enter_context(nc.semaphore("s_f"))      # ACT f ops done
        xt = ctx.enter_context(nc.sbuf_tensor("xt", [P, 3 * FMAX], fp32))
        tw = ctx.enter_context(nc.sbuf_tensor("tw", [P, 3 * FMAX], fp32))
        tc = ctx.enter_context(nc.sbuf_tensor("tc", [P, 3 * FMAX], fp32))
        ts = ctx.enter_context(nc.sbuf_tensor("ts", [P, 3 * FMAX], int32))
        to = ctx.enter_context(nc.sbuf_tensor("to", [P, 6 * FMAX], int8))

        def sl(t, j, n, ns=3):
            return t[:, (j % ns) * FMAX:(j % ns) * FMAX + n]

        def dr(t, i):
            return t[:, OFFS[i]:OFFS[i] + CHUNKS[i]]

        @block.sync
        def _(sync):
            for i in range(NCH):
                if i >= 3:
                    sync.wait_ge(s_w, i - 2)          # DVE done reading xt slot
                sync.dma_start(
                    out=sl(xt, i, CHUNKS[i]), in_=dr(xin, i)
                ).then_inc(s_in[i % 3], 16)

        @block.vector
        def _(vector):
            for ii in range(NCH + 2):
                if ii < NCH:
                    vector.wait_ge(s_in[ii % 3], 16 * (ii // 3 + 1))
                    if ii >= 3:
                        vector.wait_ge(s_c2, ii - 2)  # c done reading tw slot
                    vector.tensor_scalar(
                        out=sl(tw, ii, CHUNKS[ii]), in0=sl(xt, ii, CHUNKS[ii]),
                        scalar1=0.03125, scalar2=-0.015625,
                        op0=alu.add, op1=alu.add,
                    ).then_inc(s_w, 1)
                if 1 <= ii <= NCH:
                    i = ii - 1
                    vector.wait_ge(s_w, i + 1)        # own w(i) committed (RAW tw)
                    if i >= 3:
                        vector.wait_ge(s_s, i - 2)    # s done reading tc slot
                    vector.tensor_scalar(
                        out=sl(tc, i, CHUNKS[i]), in0=sl(tw, i, CHUNKS[i]),
                        scalar1=M5, scalar2=None, op0=alu.add,
                    ).then_inc(s_c2, 1)
                if ii >= 2:
                    k = ii - 2
                    vector.wait_ge(s_c2, k + 1)       # own c(k) committed (RAW tc)
                    if k >= 3:
                        vector.wait_ge(s_f, k - 2)    # ACT done reading ts slot
                    vector.tensor_scalar(
                        out=sl(ts, k, CHUNKS[k]),
                        in0=sl(tc, k, CHUNKS[k]).bitcast(mybir.dt.int32),
                        scalar1=1, scalar2=None,
                        op0=alu.arith_shift_right,
                    ).then_inc(s_s, 1)

        @block.scalar
        def _(scalar):
            for i in range(NCH):
                scalar.wait_ge(s_s, i + 1)
                if i >= 6:
                    scalar.wait_ge(s_out[i % 6], 16 * (i // 6))
                # ts bits = 0x24600000 + a -> fp32 value 1.75*2^-55 + a*2^-78.
                # Rebias in fp: (in * 2^78) - 1.75*2^23 = a; int8 convert is
                # exact and its saturation implements the reference clip.
                scalar.activation(
                    out=sl(to, i, CHUNKS[i], 6),
                    in_=sl(ts, i, CHUNKS[i]).bitcast(mybir.dt.float32),
                    func=act.Copy, bias=REBIAS, scale=SCALE78,
                ).then_inc(s_f, 1)
                scalar.wait_ge(s_f, i + 1)            # own f(i) committed
                scalar.dma_start(
                    out=dr(yout, i), in_=sl(to, i, CHUNKS[i], 6)
                ).then_inc(s_out[i % 6], 16)

    return nc


def kernel(x: np.ndarray) -> np.ndarray:
    from concourse.bass_utils import run_bass_kernel_spmd

    if "nc" not in _cache:
        _cache["nc"] = _build()
    nc = _cache["nc"]

    xs = np.ascontiguousarray(x, dtype=np.float32).reshape(N_CORES, P, TOT)
    in_maps = [{"x": xs[c]} for c in range(N_CORES)]

    trace = bool(os.environ.get("BASS_TRACE"))
    tmpdir = os.environ.get("BASS_TRACE_DIR") or None
    res = run_bass_kernel_spmd(
        nc, in_maps, list(range(N_CORES)), trace=trace, tmpdir=tmpdir
    )
    if res.exec_time_ns is not None:
        print(f"HW exec time: {res.exec_time_ns} ns")

    out = np.concatenate(
        [np.asarray(res.results[c]["y"]).reshape(-1) for c in range(N_CORES)]
    )
    # a * 2^-4, exact pow2 scale; int8 saturation already applied the clip.
    out = out.astype(np.float32) * np.float32(0.0625)
    return out.reshape(B, C, H, W)


# revision 6
# speedup vs baseline: 1.0178x; 1.0178x over previous
"""LinearQuant kernel for Trainium2 (8 NeuronCores, data parallel).

Reference math (fp32, bit-exact):
    delta = 2^-4; bound = 128
    out = clip(floor(x/delta + 0.5), -128, 127) * delta

Computed on-device with ONLY tensor_scalar-class ops (TT/STT ops measured
~4.5x slower than 2x-mode TS on this hardware, so the classic
RNE+compare-fixup floor was redesigned into an integer-domain floor):

  w = fl(fl(x + 2^-5) - 2^-6)        # u = fl(x+2^-5) = fl(16x+.5)/16 (pow2
                                     # scaling commutes with rounding); the
                                     # -2^-6 bias is EXACT for |u| <= 8
                                     # (span fits 24-bit mantissa)
  c = fl(w + 1.5*2^18)               # magic: c's low bits = K + k where
                                     # k = RNE(32w) = RNE(2y-0.5), ties-even
  s = c.bits >> 1                    # floor(v) == RNE(2v-0.5) >> 1 exactly
                                     # (incl. ties & negatives)
  a = s.bits_as_fp32 * 2^78 - 1.75*2^23  # -> the int index a = floor(16x+.5),
                                     # int8 out (ACT engine)

s.bits = 0x24600000 + a, i.e. fp32 value 1.75*2^-55 + a*2^-78; the *2^78
- 14680064 rebias is exact (pow2 scale; the subtract lands in [2^23,2^24)
where the grid is 1.0, and the result a is fp32-representable). The fp32
-> int8 output conversion of the exact integer a is exact, and its
saturation at [-128, 127] IS the reference's post-floor clip, so the
kernel is bit-exact for arbitrary inputs (verified on HW incl. ties and
out-of-range values). Storing int8 instead of bf16 halves store traffic:
per-core HBM is 25.7 MB in + 6.4 MB out, ~90us at the 358 GB/s
HBM-per-core limit -- the roofline. The host converts with a*2^-4
(exact pow2 scale).

Engine split: DVE runs w/c/s as fused TS ops (2x_2P mode, 2 elem/cyc
fp32); ACT runs the final rebias->int8 and the out-DMA triggers (own
HWDGE ring, so out never blocks the in stream); SP(sync) runs the
in-DMAs. Raw Block style with explicit semaphores (Tile's auto-sems hit
walrus "Too many sync wait commands" on this shape). The DVE stream is
software-pipelined (w(i), c(i-1), s(i-2)) over ring buffers.

Layout: each core's 6,422,528-elem flat slice is viewed as [128, 50176]
partition-major (a free reshape on host and an elementwise-consistent
relabeling on device). Chunks are column ranges, TAPERED: small chunks
at the head (so the first DVE op starts ~3us in instead of waiting for
a full 1.8 MB tile) and at the tail (short final act+store drain), 3584
in the middle. The xt ring has 4 slots for deeper DMA prefetch.
"""

import os

import numpy as np

B, C, H, W = 64, 256, 56, 56
N_CORES = 8
P = 128            # partitions
TOT = 50176        # free elems per partition per core
FMAX = 3584        # ring slot width (elems)
CHUNKS = [256, 256, 512, 1024, 2048] + [3584] * 12 + [2048, 1024]
assert sum(CHUNKS) == TOT and max(CHUNKS) == FMAX
NCH = len(CHUNKS)
OFFS = [sum(CHUNKS[:i]) for i in range(NCH)]
M5 = 393216.0        # 1.5*2^18: RNE-magic for the 2^-5 grid
REBIAS = -14680064.0  # -1.75*2^23
SCALE78 = float(2.0 ** 78)

_cache = {}


def _build():
    from contextlib import ExitStack

    import concourse.mybir as mybir
    from concourse.bass import Bass

    fp32 = mybir.dt.float32
    int8 = mybir.dt.int8
    int32 = mybir.dt.int32
    alu = mybir.AluOpType
    act = mybir.ActivationFunctionType

    nc = Bass()
    xin = nc.declare_dram_parameter("x", [P, TOT], fp32, isOutput=False)
    yout = nc.declare_dram_parameter("y", [P, TOT], int8, isOutput=True)

    with ExitStack() as ctx:
        block = ctx.enter_context(nc.Block())
        s_in = [ctx.enter_context(nc.semaphore(f"s_in{j}")) for j in range(3)]
        s_out = [ctx.enter_context(nc.semaphore(f"s_out{j}")) for j in range(6)]
        s_w = ctx.enter_context(nc.semaphore("s_w"))      # DVE w ops done
        s_c2 = ctx.enter_context(nc.semaphore("s_c2"))    # DVE c ops done
        s_s = ctx.enter_context(nc.semaphore("s_s"))      # DVE s ops done
        s_f = ctx.enter_context(nc.semaphore("s_f"))      # ACT f ops done
        xt = ctx.enter_context(nc.sbuf_tensor("xt", [P, 3 * FMAX], fp32))
        tw = ctx.enter_context(nc.sbuf_tensor("tw", [P, 3 * FMAX], fp32))
        tc = ctx.enter_context(nc.sbuf_tensor("tc", [P, 3 * FMAX], fp32))
        ts = ctx.enter_context(nc.sbuf_tensor("ts", [P, 3 * FMAX], int32))
        to = ctx.enter_context(nc.sbuf_tensor("to", [P, 6 * FMAX], int8))

        def sl(t, j, n, ns=3):
            return t[:, (j % ns) * FMAX:(j % ns) * FMAX + n]

        def dr(t, i):
            return t[:, OFFS[i]:OFFS[i] + CHUNKS[i]]

        @block.sync
        def _(sync):
            for i in range(NCH):
                if i >= 3:
                    sync.wait_ge(s_w, i - 2)          # DVE done reading xt slot
                sync.dma_start(
                    out=sl(xt, i, CHUNKS[i]), in_=dr(xin, i)
                ).then_inc(s_in[i % 3], 16)

        @block.vector
        def _(vector):
            for ii in range(NCH + 2):
                if ii < NCH:
                    vector.wait_ge(s_in[ii % 3], 16 * (ii // 3 + 1))
                    if ii >= 3:
                        vector.wait_ge(s_c2, ii - 2)  # c done reading tw slot
                    vector.tensor_scalar(
                        out=sl(tw, ii, CHUNKS[ii]), in0=sl(xt, ii, CHUNKS[ii]),
                        scalar1=0.03125, scalar2=-0.015625,
                        op0=alu.add, op1=alu.add,
                    ).then_inc(s_w, 1)
                if 1 <= ii <= NCH:
                    i = ii - 1
                    vector.wait_ge(s_w, i + 1)        # own w(i) committed (RAW tw)
                    if i >= 3:
                        vector.wait_ge(s_s, i - 2)    # s done reading tc slot
                    vector.tensor_scalar(
                        out=sl(tc, i, CHUNKS[i]), in0=sl(tw, i, CHUNKS[i]),
                        scalar1=M5, scalar2=None, op0=alu.add,
                    ).then_inc(s_c2, 1)
                if ii >= 2:
                    k = ii - 2
                    vector.wait_ge(s_c2, k + 1)       # own c(k) committed (RAW tc)
                    if k >= 3:
                        vector.wait_ge(s_f, k - 2)    # ACT done reading ts slot
                    vector.tensor_scalar(
                        out=sl(ts, k, CHUNKS[k]),
                        in0=sl(tc, k, CHUNKS[k]).bitcast(mybir.dt.int32),
                        scalar1=1, scalar2=None,
                        op0=alu.arith_shift_right,
                    ).then_inc(s_s, 1)

        @block.scalar
        def _(scalar):
            for i in range(NCH):
                scalar.wait_ge(s_s, i + 1)
                if i >= 6:
                    scalar.wait_ge(s_out[i % 6], 16 * (i // 6))
                # ts bits = 0x24600000 + a -> fp32 value 1.75*2^-55 + a*2^-78.
                # Rebias in fp: (in * 2^78) - 1.75*2^23 = a; int8 convert is
                # exact and its saturation implements the reference clip.
                scalar.activation(
                    out=sl(to, i, CHUNKS[i], 6),
                    in_=sl(ts, i, CHUNKS[i]).bitcast(mybir.dt.float32),
                    func=act.Copy, bias=REBIAS, scale=SCALE78,
                ).then_inc(s_f, 1)
                scalar.wait_ge(s_f, i + 1)            # own f(i) committed
                scalar.dma_start(
                    out=dr(yout, i), in_=sl(to, i, CHUNKS[i], 6)
                ).then_inc(s_out[i % 6], 16)

    return nc


def kernel(x: np.ndarray) -> np.ndarray:
    from concourse.bass_utils import run_bass_kernel_spmd

    if "nc" not in _cache:
        _cache["nc"] = _build()
    nc = _cache["nc"]

    xs = np.ascontiguousarray(x, dtype=np.float32).reshape(N_CORES, P, TOT)
    in_maps = [{"x": xs[c]} for c in range(N_CORES)]

    trace = bool(os.environ.get("BASS_TRACE"))
    tmpdir = os.environ.get("BASS_TRACE_DIR") or None
    res = run_bass_kernel_spmd(
        nc, in_maps, list(range(N_CORES)), trace=trace, tmpdir=tmpdir
    )
    if res.exec_time_ns is not None:
        print(f"HW exec time: {res.exec_time_ns} ns")

    out = np.concatenate(
        [np.asarray(res.results[c]["y"]).reshape(-1) for c in range(N_CORES)]
    )
    # a * 2^-4, exact pow2 scale; int8 saturation already applied the clip.
    out = out.astype(np.float32) * np.float32(0.0625)
    return out.reshape(B, C, H, W)


# revision 7
# speedup vs baseline: 1.1204x; 1.1008x over previous
"""LinearQuant kernel for Trainium2 (8 NeuronCores, data parallel).

Reference math (fp32, bit-exact):
    delta = 2^-4; bound = 128
    out = clip(floor(x/delta + 0.5), -128, 127) * delta

Computed on-device with ONLY tensor_scalar-class ops (TT/STT ops measured
~4.5x slower than 2x-mode TS on this hardware, so the classic
RNE+compare-fixup floor was redesigned into an integer-domain floor):

  w = fl(fl(x + 2^-5) - 2^-6)        # u = fl(x+2^-5) = fl(16x+.5)/16 (pow2
                                     # scaling commutes with rounding); the
                                     # -2^-6 bias is EXACT for |u| <= 8
                                     # (span fits 24-bit mantissa)
  c = fl(w + 1.5*2^18)               # magic: c's low bits = K + k where
                                     # k = RNE(32w) = RNE(2y-0.5), ties-even
  s = c.bits >> 1                    # floor(v) == RNE(2v-0.5) >> 1 exactly
                                     # (incl. ties & negatives)
  a = s.bits_as_fp32 * 2^78 - 1.75*2^23  # -> the int index a = floor(16x+.5),
                                     # int8 out (ACT engine)

s.bits = 0x24600000 + a, i.e. fp32 value 1.75*2^-55 + a*2^-78; the *2^78
- 14680064 rebias is exact (pow2 scale; the subtract lands in [2^23,2^24)
where the grid is 1.0, and the result a is fp32-representable). The fp32
-> int8 output conversion of the exact integer a is exact, and its
saturation at [-128, 127] IS the reference's post-floor clip, so the
kernel is bit-exact for arbitrary inputs (verified on HW incl. ties and
out-of-range values). Storing int8 instead of bf16 halves store traffic:
per-core HBM is 25.7 MB in + 6.4 MB out, ~90us at the 358 GB/s
HBM-per-core limit -- the roofline. The host converts with a*2^-4
(exact pow2 scale).

Engine split: DVE runs w/c/s as fused TS ops (2x_2P mode, 2 elem/cyc
fp32); ACT runs the final rebias->int8 and the out-DMA triggers (own
HWDGE ring, so out never blocks the in stream); SP(sync) runs the
in-DMAs. Raw Block style with explicit semaphores (Tile's auto-sems hit
walrus "Too many sync wait commands" on this shape). The DVE stream is
software-pipelined (w(i), c(i-1), s(i-2)) over ring buffers.

Layout: each core's 6,422,528-elem flat slice is viewed as [128, 50176]
partition-major (a free reshape on host and an elementwise-consistent
relabeling on device).

Granularity is DECOUPLED between input and output:
 - input chunks (DVE w/c/s granularity) are TAPERED: tiny at the head so
   the first DVE op starts ~3us in (not after a full 1.8 MB tile), and
   split at the tail so the last dependency chain is short;
 - output segments (ACT rebias + out-DMA granularity) are uniform
   3584-elem windows (descriptors always 3584 B >= the 512 B DMA
   line-rate minimum), except the last window which is split in two
   1792-elem segments to shorten the final drain.
The s-pass writes ts at window-absolute positions so each ACT segment
reads one contiguous range.
"""

import os

import numpy as np

B, C, H, W = 64, 256, 56, 56
N_CORES = 8
P = 128            # partitions
TOT = 50176        # free elems per partition per core
FMAX = 3584        # ring slot width / output window (elems)
# input chunks (offset, len): head taper fills window 0, then 12 full
# windows, then window 13 split 2048+1536.
IN_CHUNKS = (
    [(0, 256), (256, 256), (512, 512), (1024, 1024), (2048, 1536)]
    + [(j * FMAX, FMAX) for j in range(1, 13)]
    + [(46592, 2048), (48640, 1536)]
)
NCH = len(IN_CHUNKS)
assert IN_CHUNKS[-1][0] + IN_CHUNKS[-1][1] == TOT
# output segments (offset, len, last-covering-chunk-idx)
SEGS = [(j * FMAX, FMAX, 4 + j) for j in range(13)] + [
    (46592, 1792, 17),
    (48384, 1792, 18),
]
NSEG = len(SEGS)
M5 = 393216.0        # 1.5*2^18: RNE-magic for the 2^-5 grid
REBIAS = -14680064.0  # -1.75*2^23
SCALE78 = float(2.0 ** 78)

_cache = {}


def _build():
    from contextlib import ExitStack

    import concourse.mybir as mybir
    from concourse.bass import Bass

    fp32 = mybir.dt.float32
    int8 = mybir.dt.int8
    int32 = mybir.dt.int32
    alu = mybir.AluOpType
    act = mybir.ActivationFunctionType

    nc = Bass()
    xin = nc.declare_dram_parameter("x", [P, TOT], fp32, isOutput=False)
    yout = nc.declare_dram_parameter("y", [P, TOT], int8, isOutput=True)

    with ExitStack() as ctx:
        block = ctx.enter_context(nc.Block())
        s_in = [ctx.enter_context(nc.semaphore(f"s_in{j}")) for j in range(3)]
        s_out = [ctx.enter_context(nc.semaphore(f"s_out{j}")) for j in range(6)]
        s_w = ctx.enter_context(nc.semaphore("s_w"))      # DVE w ops done
        s_c2 = ctx.enter_context(nc.semaphore("s_c2"))    # DVE c ops done
        s_s = ctx.enter_context(nc.semaphore("s_s"))      # DVE s ops done
        s_f = ctx.enter_context(nc.semaphore("s_f"))      # ACT f segs done
        xt = ctx.enter_context(nc.sbuf_tensor("xt", [P, 3 * FMAX], fp32))
        tw = ctx.enter_context(nc.sbuf_tensor("tw", [P, 3 * FMAX], fp32))
        tc = ctx.enter_context(nc.sbuf_tensor("tc", [P, 3 * FMAX], fp32))
        ts = ctx.enter_context(nc.sbuf_tensor("ts", [P, 3 * FMAX], int32))
        to = ctx.enter_context(nc.sbuf_tensor("to", [P, 6 * FMAX], int8))

        def sl(t, j, n, ns=3):
            # chunk-indexed ring slot (xt/tw/tc)
            return t[:, (j % ns) * FMAX:(j % ns) * FMAX + n]

        def wsl(t, off, n, ns):
            # window-absolute ring position (ts/to)
            w = off // FMAX
            base = (w % ns) * FMAX + (off - w * FMAX)
            return t[:, base:base + n]

        @block.sync
        def _(sync):
            for i, (off, n) in enumerate(IN_CHUNKS):
                if i >= 3:
                    sync.wait_ge(s_w, i - 2)          # DVE done reading xt slot
                sync.dma_start(
                    out=sl(xt, i, n), in_=xin[:, off:off + n]
                ).then_inc(s_in[i % 3], 16)

        @block.vector
        def _(vector):
            for ii in range(NCH + 2):
                if ii < NCH:
                    n = IN_CHUNKS[ii][1]
                    vector.wait_ge(s_in[ii % 3], 16 * (ii // 3 + 1))
                    if ii >= 3:
                        vector.wait_ge(s_c2, ii - 2)  # c done reading tw slot
                    vector.tensor_scalar(
                        out=sl(tw, ii, n), in0=sl(xt, ii, n),
                        scalar1=0.03125, scalar2=-0.015625,
                        op0=alu.add, op1=alu.add,
                    ).then_inc(s_w, 1)
                if 1 <= ii <= NCH:
                    i = ii - 1
                    n = IN_CHUNKS[i][1]
                    vector.wait_ge(s_w, i + 1)        # own w(i) committed (RAW tw)
                    if i >= 3:
                        vector.wait_ge(s_s, i - 2)    # s done reading tc slot
                    vector.tensor_scalar(
                        out=sl(tc, i, n), in0=sl(tw, i, n),
                        scalar1=M5, scalar2=None, op0=alu.add,
                    ).then_inc(s_c2, 1)
                if ii >= 2:
                    k = ii - 2
                    off, n = IN_CHUNKS[k]
                    vector.wait_ge(s_c2, k + 1)       # own c(k) committed (RAW tc)
                    win = off // FMAX
                    if win >= 3:
                        # ACT read all segs of window win-3 (1 seg per window
                        # for windows 0..12): s_f >= win-2
                        vector.wait_ge(s_f, win - 2)
                    vector.tensor_scalar(
                        out=wsl(ts, off, n, 3),
                        in0=sl(tc, k, n).bitcast(mybir.dt.int32),
                        scalar1=1, scalar2=None,
                        op0=alu.arith_shift_right,
                    ).then_inc(s_s, 1)

        @block.scalar
        def _(scalar):
            for si, (off, n, dep) in enumerate(SEGS):
                win = off // FMAX
                scalar.wait_ge(s_s, dep + 1)
                if win >= 6 and off % FMAX == 0:
                    scalar.wait_ge(s_out[win % 6], 16 * (win // 6))
                # ts bits = 0x24600000 + a -> fp32 value 1.75*2^-55 + a*2^-78.
                # Rebias in fp: (in * 2^78) - 1.75*2^23 = a; int8 convert is
                # exact and its saturation implements the reference clip.
                scalar.activation(
                    out=wsl(to, off, n, 6),
                    in_=wsl(ts, off, n, 3).bitcast(mybir.dt.float32),
                    func=act.Copy, bias=REBIAS, scale=SCALE78,
                ).then_inc(s_f, 1)
                scalar.wait_ge(s_f, si + 1)           # own f(si) committed
                scalar.dma_start(
                    out=yout[:, off:off + n], in_=wsl(to, off, n, 6)
                ).then_inc(s_out[win % 6], 16)

    return nc


def kernel(x: np.ndarray) -> np.ndarray:
    from concourse.bass_utils import run_bass_kernel_spmd

    if "nc" not in _cache:
        _cache["nc"] = _build()
    nc = _cache["nc"]

    xs = np.ascontiguousarray(x, dtype=np.float32).reshape(N_CORES, P, TOT)
    in_maps = [{"x": xs[c]} for c in range(N_CORES)]

    trace = bool(os.environ.get("BASS_TRACE"))
    tmpdir = os.environ.get("BASS_TRACE_DIR") or None
    res = run_bass_kernel_spmd(
        nc, in_maps, list(range(N_CORES)), trace=trace, tmpdir=tmpdir
    )
    if res.exec_time_ns is not None:
        print(f"HW exec time: {res.exec_time_ns} ns")

    out = np.concatenate(
        [np.asarray(res.results[c]["y"]).reshape(-1) for c in range(N_CORES)]
    )
    # a * 2^-4, exact pow2 scale; int8 saturation already applied the clip.
    out = out.astype(np.float32) * np.float32(0.0625)
    return out.reshape(B, C, H, W)


# revision 8
# speedup vs baseline: 1.1425x; 1.0197x over previous
"""LinearQuant kernel for Trainium2 (8 NeuronCores, data parallel).

Reference math (fp32, bit-exact):
    delta = 2^-4; bound = 128
    out = clip(floor(x/delta + 0.5), -128, 127) * delta

Computed on-device with ONLY tensor_scalar-class ops (TT/STT ops measured
~4.5x slower than 2x-mode TS on this hardware, so the classic
RNE+compare-fixup floor was redesigned into an integer-domain floor):

  w = fl(fl(x + 2^-5) - 2^-6)        # u = fl(x+2^-5) = fl(16x+.5)/16 (pow2
                                     # scaling commutes with rounding); the
                                     # -2^-6 bias is EXACT for |u| <= 8
                                     # (span fits 24-bit mantissa)
  c = fl(w + 1.5*2^18)               # magic: c's low bits = K + k where
                                     # k = RNE(32w) = RNE(2y-0.5), ties-even
  s = c.bits >> 1                    # floor(v) == RNE(2v-0.5) >> 1 exactly
                                     # (incl. ties & negatives)
  a = s.bits_as_fp32 * 2^78 - 1.75*2^23  # -> the int index a = floor(16x+.5),
                                     # int8 out (ACT engine)

s.bits = 0x24600000 + a, i.e. fp32 value 1.75*2^-55 + a*2^-78; the *2^78
- 14680064 rebias is exact (pow2 scale; the subtract lands in [2^23,2^24)
where the grid is 1.0, and the result a is fp32-representable). The fp32
-> int8 output conversion of the exact integer a is exact, and its
saturation at [-128, 127] IS the reference's post-floor clip, so the
kernel is bit-exact for arbitrary inputs (verified on HW incl. ties and
out-of-range values). Storing int8 instead of bf16 halves store traffic:
per-core HBM is 25.7 MB in + 6.4 MB out, ~90us at the 358 GB/s
HBM-per-core limit -- the roofline. The host converts with a*2^-4
(exact pow2 scale).

Engine split: DVE runs w/c/s as fused TS ops (2x_2P mode, 2 elem/cyc
fp32); ACT runs the final rebias->int8 and the out-DMA triggers (own
HWDGE ring, so out never blocks the in stream); SP(sync) runs the
in-DMAs. Raw Block style with explicit semaphores (Tile's auto-sems hit
walrus "Too many sync wait commands" on this shape). The DVE stream is
software-pipelined (w(i), c(i-1), s(i-2)) over ring buffers.

Layout: each core's 6,422,528-elem flat slice is viewed as [128, 50176]
partition-major (a free reshape on host and an elementwise-consistent
relabeling on device).

Granularity is DECOUPLED between input and output:
 - input chunks (DVE w/c/s granularity) are TAPERED: tiny at the head so
   the first DVE op starts ~3us in (not after a full 1.8 MB tile), and
   split at the tail so the last dependency chain is short;
 - output segments (ACT rebias + out-DMA granularity) are uniform
   3584-elem windows (descriptors always 3584 B >= the 512 B DMA
   line-rate minimum), except the last window which is split in two
   1792-elem segments to shorten the final drain.
The s-pass writes ts at window-absolute positions so each ACT segment
reads one contiguous range.
"""

import os

import numpy as np

B, C, H, W = 64, 256, 56, 56
N_CORES = 8
P = 128            # partitions
TOT = 50176        # free elems per partition per core
FMAX = 3584        # ring slot width / output window (elems)
# input chunks (offset, len): head taper fills window 0, then 12 full
# windows, then window 13 split 2048+1536.
IN_CHUNKS = (
    [(0, 256), (256, 256), (512, 512), (1024, 1024), (2048, 1536)]
    + [(3584, 1792), (5376, 1792)]
    + [(j * FMAX, FMAX) for j in range(2, 13)]
    + [(46592, 2048), (48640, 1536)]
)
NCH = len(IN_CHUNKS)
assert IN_CHUNKS[-1][0] + IN_CHUNKS[-1][1] == TOT


def _dep(off, n):
    # last input chunk overlapping [off, off+n)
    return max(i for i, (o, m) in enumerate(IN_CHUNKS) if o < off + n and o + m > off)


# output segments (offset, len, last-covering-chunk-idx). Uniform 3584
# windows; the final window is split to shorten the drain. The last
# N_SYNC_TRIG segments' out-DMAs are triggered by the sync engine (idle
# after the in stream) so ACT's tail is pure back-to-back activations.
SEGS = [(j * FMAX, FMAX, _dep(j * FMAX, FMAX)) for j in range(13)] + [
    (46592, 1792, _dep(46592, 1792)),
    (48384, 1792, _dep(48384, 1792)),
]
NSEG = len(SEGS)
N_SYNC_TRIG = 3
M5 = 393216.0        # 1.5*2^18: RNE-magic for the 2^-5 grid
REBIAS = -14680064.0  # -1.75*2^23
SCALE78 = float(2.0 ** 78)

_cache = {}


def _build():
    from contextlib import ExitStack

    import concourse.mybir as mybir
    from concourse.bass import Bass

    fp32 = mybir.dt.float32
    int8 = mybir.dt.int8
    int32 = mybir.dt.int32
    alu = mybir.AluOpType
    act = mybir.ActivationFunctionType

    nc = Bass()
    xin = nc.declare_dram_parameter("x", [P, TOT], fp32, isOutput=False)
    yout = nc.declare_dram_parameter("y", [P, TOT], int8, isOutput=True)

    with ExitStack() as ctx:
        block = ctx.enter_context(nc.Block())
        s_in = [ctx.enter_context(nc.semaphore(f"s_in{j}")) for j in range(3)]
        s_out = [ctx.enter_context(nc.semaphore(f"s_out{j}")) for j in range(6)]
        s_w = ctx.enter_context(nc.semaphore("s_w"))      # DVE w ops done
        s_c2 = ctx.enter_context(nc.semaphore("s_c2"))    # DVE c ops done
        s_s = ctx.enter_context(nc.semaphore("s_s"))      # DVE s ops done
        s_f = ctx.enter_context(nc.semaphore("s_f"))      # ACT f segs done
        xt = ctx.enter_context(nc.sbuf_tensor("xt", [P, 3 * FMAX], fp32))
        tw = ctx.enter_context(nc.sbuf_tensor("tw", [P, 3 * FMAX], fp32))
        tc = ctx.enter_context(nc.sbuf_tensor("tc", [P, 3 * FMAX], fp32))
        ts = ctx.enter_context(nc.sbuf_tensor("ts", [P, 3 * FMAX], int32))
        to = ctx.enter_context(nc.sbuf_tensor("to", [P, 6 * FMAX], int8))

        def sl(t, j, n, ns=3):
            # chunk-indexed ring slot (xt/tw/tc)
            return t[:, (j % ns) * FMAX:(j % ns) * FMAX + n]

        def wsl(t, off, n, ns):
            # window-absolute ring position (ts/to)
            w = off // FMAX
            base = (w % ns) * FMAX + (off - w * FMAX)
            return t[:, base:base + n]

        @block.sync
        def _(sync):
            for i, (off, n) in enumerate(IN_CHUNKS):
                if i >= 3:
                    sync.wait_ge(s_w, i - 2)          # DVE done reading xt slot
                sync.dma_start(
                    out=sl(xt, i, n), in_=xin[:, off:off + n]
                ).then_inc(s_in[i % 3], 16)
            for si in range(NSEG - N_SYNC_TRIG, NSEG):
                off, n, _ = SEGS[si]
                win = off // FMAX
                sync.wait_ge(s_f, si + 1)
                sync.dma_start(
                    out=yout[:, off:off + n], in_=wsl(to, off, n, 6)
                ).then_inc(s_out[win % 6], 16)

        @block.vector
        def _(vector):
            for ii in range(NCH + 2):
                if ii < NCH:
                    n = IN_CHUNKS[ii][1]
                    vector.wait_ge(s_in[ii % 3], 16 * (ii // 3 + 1))
                    if ii >= 3:
                        vector.wait_ge(s_c2, ii - 2)  # c done reading tw slot
                    vector.tensor_scalar(
                        out=sl(tw, ii, n), in0=sl(xt, ii, n),
                        scalar1=0.03125, scalar2=-0.015625,
                        op0=alu.add, op1=alu.add,
                    ).then_inc(s_w, 1)
                if 1 <= ii <= NCH:
                    i = ii - 1
                    n = IN_CHUNKS[i][1]
                    vector.wait_ge(s_w, i + 1)        # own w(i) committed (RAW tw)
                    if i >= 3:
                        vector.wait_ge(s_s, i - 2)    # s done reading tc slot
                    vector.tensor_scalar(
                        out=sl(tc, i, n), in0=sl(tw, i, n),
                        scalar1=M5, scalar2=None, op0=alu.add,
                    ).then_inc(s_c2, 1)
                if ii >= 2:
                    k = ii - 2
                    off, n = IN_CHUNKS[k]
                    vector.wait_ge(s_c2, k + 1)       # own c(k) committed (RAW tc)
                    win = off // FMAX
                    if win >= 3:
                        # ACT read all segs of window win-3 (1 seg per window
                        # for windows 0..12): s_f >= win-2
                        vector.wait_ge(s_f, win - 2)
                    vector.tensor_scalar(
                        out=wsl(ts, off, n, 3),
                        in0=sl(tc, k, n).bitcast(mybir.dt.int32),
                        scalar1=1, scalar2=None,
                        op0=alu.arith_shift_right,
                    ).then_inc(s_s, 1)

        @block.scalar
        def _(scalar):
            for si, (off, n, dep) in enumerate(SEGS):
                win = off // FMAX
                scalar.wait_ge(s_s, dep + 1)
                if win >= 6 and off % FMAX == 0:
                    scalar.wait_ge(s_out[win % 6], 16 * (win // 6))
                # ts bits = 0x24600000 + a -> fp32 value 1.75*2^-55 + a*2^-78.
                # Rebias in fp: (in * 2^78) - 1.75*2^23 = a; int8 convert is
                # exact and its saturation implements the reference clip.
                scalar.activation(
                    out=wsl(to, off, n, 6),
                    in_=wsl(ts, off, n, 3).bitcast(mybir.dt.float32),
                    func=act.Copy, bias=REBIAS, scale=SCALE78,
                ).then_inc(s_f, 1)
                if si < NSEG - N_SYNC_TRIG:
                    scalar.wait_ge(s_f, si + 1)       # own f(si) committed
                    scalar.dma_start(
                        out=yout[:, off:off + n], in_=wsl(to, off, n, 6)
                    ).then_inc(s_out[win % 6], 16)

    return nc


def kernel(x: np.ndarray) -> np.ndarray:
    from concourse.bass_utils import run_bass_kernel_spmd

    if "nc" not in _cache:
        _cache["nc"] = _build()
    nc = _cache["nc"]

    xs = np.ascontiguousarray(x, dtype=np.float32).reshape(N_CORES, P, TOT)
    in_maps = [{"x": xs[c]} for c in range(N_CORES)]

    trace = bool(os.environ.get("BASS_TRACE"))
    tmpdir = os.environ.get("BASS_TRACE_DIR") or None
    res = run_bass_kernel_spmd(
        nc, in_maps, list(range(N_CORES)), trace=trace, tmpdir=tmpdir
    )
    if res.exec_time_ns is not None:
        print(f"HW exec time: {res.exec_time_ns} ns")

    out = np.concatenate(
        [np.asarray(res.results[c]["y"]).reshape(-1) for c in range(N_CORES)]
    )
    # a * 2^-4, exact pow2 scale; int8 saturation already applied the clip.
    out = out.astype(np.float32) * np.float32(0.0625)
    return out.reshape(B, C, H, W)
